# revision 3
# baseline (speedup 1.0000x reference)
"""Cross-attention kernel for TRN2, 8-core SPMD.

Reference op (B=4, T=2048, S=512, D=1024, H=16, Hd=64):
    q = (x @ Wq + bq); k,v = context @ Wkv + bkv
    out = softmax(q k^T / sqrt(Hd) + mask) @ v @ Wp + bp

Sharding: pure data-parallel over (batch, T/2): core c owns batch c//2,
query rows (c%2)*1024..+1024.  Each core recomputes K/V for its batch
(2x duplicated KV-proj work, zero collectives).  Weights replicated.

v2 schedule (per core, R=1024 query rows).  The ACT exp stream (64 x
[128,1024] Exp, ~1.1us each) is the pacing engine, so the kernel is
restructured as a per-head-pair software pipeline that starts it ASAP
and never blocks it:
  - head-streamed: q_proj(m) -> k_proj(m) -> QK(m) + exps, with
    av(m-1) and the two v_proj halves woven in, so the first Exp fires
    ~10us in (vs ~51us for phase-ordered) and ACT runs continuously.
  - ALL softmax reciprocals are deferred to one batched ACT phase after
    the last Exp (sums staged to SBUF in fp16): one Exp->Recip LUT
    switch total, instead of 12 table loads thrashing mid-stream (each
    1.3us, and the blockage used to idle the PE long enough for HAM to
    re-throttle it to 1.2GHz for 24us).
  - input DMAs spread over all three DGE queues (sync/SP, scalar/ACT,
    gpsimd) so q_proj's operands land by ~8us; y output DMAs alternate
    sync/gpsimd to halve the drain tail.

Device design otherwise as v1: all activations flow transposed
(feature on partitions) so contractions land on partitions with no
on-chip transposes (host pre-transposes x/context):
  - KT [D,S], QT [D,R] fp16; V_aug fp16 [S, 8 pairs x 192] =
    [V_even(64)|ones(64)|V_odd(64)], wv and ones pre-scaled 2^-10 so
    unnormalized attention outputs fit fp16.
  - scores^T [S,R] per head: K=64 fp16 matmuls, two heads in PE
    row-groups 0/64 (concurrent), one ACT Exp per [128,1024] tile with
    the context mask as per-partition bias; no max-subtraction
    (|scores| <= ~8; fp32 exp is safe).
  - AV: fp16 matmuls -> psum [128,512]; the ones columns emit the
    softmax denominators for free; O and sums staged to SBUF by DVE
    (sums in fp16), batched ACT Reciprocal in-place at the end, a
    cross-partition DMA aligns each head's recip with its O rows, one
    DVE multiply normalizes OT.
  - Y [R,D] = OT^T @ Wp + bp, evictions + DMAs interleaved, rc0 rows
    first (their normalize completes first).

Numerics: fp16 operands (N(0,1)-scaled data: fp16 beats bf16 ~8x in
precision at identical PE throughput); PSUM accumulation fp32.
Max-abs error vs fp32 reference ~1e-3 of max|out|.
"""
import os
import sys
import types

import numpy as np

import concourse.tile as tile
from concourse import bacc, mybir
from concourse.bass_utils import run_bass_kernel_spmd

F32 = mybir.dt.float32
F16 = mybir.dt.float16
AF = mybir.ActivationFunctionType

B, T, S, D = 4, 2048, 512, 1024
H, HD = 16, 64
NCORE = 8
R = B * T // NCORE          # 1024 query rows per core
KC = D // 128               # 8 contraction chunks
SC = S // 128               # 4 context chunks
NP = H // 2                 # 8 head pairs
NEG = -60.0                 # mask bias (exp(-60) ~ 0)

_CACHE = {}
last_results = None         # BassKernelResults of the most recent run


def _install_ntff_hook():
    """antenv.axon_hooks is absent in this image; recreate it from the
    boot helper so BASS_TRACE=1 profiling works. Best-effort."""
    try:
        import antenv.axon_hooks  # noqa: F401
        return
    except ImportError:
        pass
    try:
        from trn_agent_boot.trn_boot import _ntff_profile_via_ctypes
        hook = _ntff_profile_via_ctypes("/opt/axon/libaxon_pjrt.so")
        mod = types.ModuleType("antenv.axon_hooks")
        mod.get_axon_ntff_profile_hook = lambda: hook
        sys.modules["antenv.axon_hooks"] = mod
    except Exception:
        pass


_install_ntff_hook()


def _act_recip(nc, out_ap, in_ap):
    """Raw ACT Reciprocal (bass blocks the helper for accuracy reasons;
    measured ~1e-5 rel err here, plenty for softmax denominators)."""
    eng = nc.scalar
    return eng.add_instruction(
        mybir.InstActivation(
            name=nc.get_next_instruction_name(),
            func=AF.Reciprocal,
            ins=[eng.lower_ap(in_ap),
                 mybir.ImmediateValue(dtype=F32, value=0.0),
                 mybir.ImmediateValue(dtype=F32, value=1.0),
                 mybir.ImmediateValue(dtype=F32, value=0.0)],
            outs=[eng.lower_ap(out_ap)],
        ))


def _build():
    nc = bacc.Bacc("TRN2", target_bir_lowering=False, debug=False,
                   num_devices=NCORE)

    xT = nc.dram_tensor("xT", [D, R], F16, kind="ExternalInput").ap()
    ctxT = nc.dram_tensor("ctxT", [D, S], F16, kind="ExternalInput").ap()
    maskb = nc.dram_tensor("maskb", [128, SC], F32, kind="ExternalInput").ap()
    wq = nc.dram_tensor("wq", [D, D], F16, kind="ExternalInput").ap()
    bq = nc.dram_tensor("bq", [128, KC], F32, kind="ExternalInput").ap()
    wk = nc.dram_tensor("wk", [D, D], F16, kind="ExternalInput").ap()
    bk = nc.dram_tensor("bk", [128, KC], F32, kind="ExternalInput").ap()
    wv = nc.dram_tensor("wv", [D, D], F16, kind="ExternalInput").ap()
    wp = nc.dram_tensor("wp", [D, D], F16, kind="ExternalInput").ap()
    bp_r = nc.dram_tensor("bp_r", [128, D], F32, kind="ExternalInput").ap()
    ones = nc.dram_tensor("ones", [128, 512], F16, kind="ExternalInput").ap()
    y = nc.dram_tensor("y", [R, D], F32, kind="ExternalOutput").ap()

    with tile.TileContext(nc) as tc:
        # Pool stack bottom -> top.  exp/psum pools sit on top so they
        # can close after av(7), making room for rcpal / psD / y.
        p_const = tc.tile_pool(name="const", bufs=1)
        p_kv = tc.tile_pool(name="kv", bufs=1)
        p_qt = tc.tile_pool(name="qt", bufs=1)
        p_ot = tc.tile_pool(name="ot", bufs=1)
        p_wp = tc.tile_pool(name="wpp", bufs=1)
        p_sums = tc.tile_pool(name="sums", bufs=1)
        p_ld = tc.tile_pool(name="ld", bufs=1)
        p_exp = tc.tile_pool(name="exp", bufs=12)
        p_psAB = tc.tile_pool(name="psAB", bufs=2, space="PSUM")
        p_psQK = tc.tile_pool(name="psQK", bufs=2, space="PSUM")
        p_psAV = tc.tile_pool(name="psAV", bufs=2, space="PSUM")
        constp = p_const.__enter__()
        kvp = p_kv.__enter__()
        qtp = p_qt.__enter__()
        otp = p_ot.__enter__()
        wpp = p_wp.__enter__()
        sumsp = p_sums.__enter__()
        ldp = p_ld.__enter__()
        expp = p_exp.__enter__()
        psAB = p_psAB.__enter__()
        psQK = p_psQK.__enter__()
        psAV = p_psAV.__enter__()

        # ---- PE warm-up on a memset tile: covers the initial DMA
        # window and gets HAM to K=8/8 before real work ----
        warm_sb = constp.tile([128, 512], F16, tag="warm_sb")
        nc.vector.memset(warm_sb[:], 0.0)
        warm_ps = psAB.tile([128, 512], F32, tag="psAB")
        for w in range(20):
            nc.tensor.matmul(warm_ps[:], warm_sb[:, 0:128], warm_sb[:],
                             start=True, stop=True, skip_group_check=True)

        # ---- input DMAs, spread over the 3 DGE queues; ordered so
        # q_proj(0)/k_proj(0) operands land first ----
        xT_t = [ldp.tile([128, R], F16, tag=f"xT{k}", name=f"xTs{k}")
                for k in range(KC)]
        wq_t = [ldp.tile([128, D], F16, tag=f"wq{k}", name=f"wqs{k}")
                for k in range(KC)]
        wk_t = [ldp.tile([128, D], F16, tag=f"wk{k}", name=f"wk{k}")
                for k in range(KC)]
        wv_t = [ldp.tile([128, D], F16, tag=f"wv{k}", name=f"wv{k}")
                for k in range(KC)]
        ctx_t = [ldp.tile([128, S], F16, tag=f"ctx{k}", name=f"ctx{k}")
                 for k in range(KC)]
        mb_t = constp.tile([128, SC], F32, tag="mb")
        bq_t = constp.tile([128, KC], F32, tag="bq")
        bk_t = constp.tile([128, KC], F32, tag="bk")
        bp_t = constp.tile([128, D], F32, tag="bp")
        wp_t = [wpp.tile([128, D], F16, tag=f"wp{k}", name=f"wps{k}")
                for k in range(KC)]

        # sync/SP queue: wq, xT[0:4], wp (ones rides in v_proj(0))
        for k in range(KC):
            nc.sync.dma_start(wq_t[k][:], wq[k * 128:(k + 1) * 128, :])
        for k in range(4):
            nc.sync.dma_start(xT_t[k][:], xT[k * 128:(k + 1) * 128, :])
        for k in range(KC):
            nc.sync.dma_start(wp_t[k][:], wp[k * 128:(k + 1) * 128, :])
        # scalar/ACT queue: biases+mask (tiny), xT[4:8], ctx
        nc.scalar.dma_start(bq_t[:], bq[:])
        nc.scalar.dma_start(bk_t[:], bk[:])
        nc.scalar.dma_start(mb_t[:], maskb[:])
        for k in range(4, KC):
            nc.scalar.dma_start(xT_t[k][:], xT[k * 128:(k + 1) * 128, :])
        for k in range(KC):
            nc.scalar.dma_start(ctx_t[k][:], ctxT[k * 128:(k + 1) * 128, :])
        # gpsimd queue: wk, wv, bp
        for k in range(KC):
            nc.gpsimd.dma_start(wk_t[k][:], wk[k * 128:(k + 1) * 128, :])
        for k in range(KC):
            nc.gpsimd.dma_start(wv_t[k][:], wv[k * 128:(k + 1) * 128, :])
        nc.gpsimd.dma_start(bp_t[:], bp_r[:])

        # ---- persistent attention operands (fp16) ----
        KT = [kvp.tile([128, S], F16, tag=f"KT{m}", name=f"KT{m}")
              for m in range(KC)]
        # V_aug: [128, pair, 192] = [V_even | ones(64) | V_odd]
        VA = [kvp.tile([128, NP, 192], F16, tag=f"VA{s}", name=f"VA{s}")
              for s in range(SC)]
        QT = [qtp.tile([128, R], F16, tag=f"QT{m}", name=f"QT{m}")
              for m in range(KC)]
        OT = [otp.tile([128, R], F16, tag=f"OT{m}", name=f"OT{m}")
              for m in range(KC)]
        # sums / rcp staging: [128, rc, hp, 512] fp16
        sums_t = sumsp.tile([128, 2, NP, 512], F16, tag="sums")

        def q_proj(m):
            for rc in range(2):
                ps = psAB.tile([128, 512], F32, tag="psAB")
                for k in range(KC):
                    nc.tensor.matmul(
                        ps[:], wq_t[k][:, m * 128:(m + 1) * 128],
                        xT_t[k][:, rc * 512:(rc + 1) * 512],
                        start=(k == 0), stop=(k == KC - 1))
                nc.vector.tensor_scalar_add(
                    QT[m][:, rc * 512:(rc + 1) * 512], ps[:], bq_t[:, m:m + 1])

        def k_proj(m):
            ps = psAB.tile([128, S], F32, tag="psAB")
            for k in range(KC):
                nc.tensor.matmul(ps[:], wk_t[k][:, m * 128:(m + 1) * 128],
                                 ctx_t[k][:],
                                 start=(k == 0), stop=(k == KC - 1))
            nc.vector.tensor_scalar_add(KT[m][:], ps[:], bk_t[:, m:m + 1])

        def v_proj(n):
            for s in range(SC):
                if n == 0:
                    nc.sync.dma_start(
                        VA[s][:, :, 64:128],
                        ones[:].rearrange("p (h c) -> p h c", c=64))
                ps = psAB.tile([128, 512], F32, tag="psAB")
                for k in range(KC):
                    nc.tensor.matmul(ps[:], ctx_t[k][:, s * 128:(s + 1) * 128],
                                     wv_t[k][:, n * 512:(n + 1) * 512],
                                     start=(k == 0), stop=(k == KC - 1))
                # scatter 8 heads (4 pairs) into V_aug blocks
                src = ps[:].rearrange("p (h c) -> p h c", c=64)
                nc.vector.tensor_copy(VA[s][:, 4 * n:4 * n + 4, 0:64],
                                      src[:, 0::2, :])
                nc.vector.tensor_copy(VA[s][:, 4 * n:4 * n + 4, 128:192],
                                      src[:, 1::2, :])

        def attn_qk(hp):
            ex = [[expp.tile([128, R], F16, tag="exp", name=f"ex{hp}_{e}_{s}")
                   for s in range(SC)] for e in range(2)]
            for s in range(SC):
                # interleave the two head row-groups so the PE runs the
                # K=64 matmuls concurrently in row-group tiles
                pss = [psQK.tile([128, R], F32, tag="psQK",
                                 name=f"psqk{hp}_{s}_{e}") for e in range(2)]
                for rc in range(2):
                    for e in range(2):
                        lo, hi = 64 * e, 64 * e + 64
                        nc.tensor.matmul(
                            pss[e][:, rc * 512:(rc + 1) * 512],
                            KT[hp][lo:hi, s * 128:(s + 1) * 128],
                            QT[hp][lo:hi, rc * 512:(rc + 1) * 512],
                            start=True, stop=True)
                for e in range(2):
                    nc.scalar.activation(ex[e][s][:], pss[e][:],
                                         AF.Exp, bias=mb_t[:, s:s + 1])
            return ex

        def attn_av(hp, ex):
            for rc in range(2):
                rr = slice(rc * 512, rc * 512 + 512)
                for e in range(2):
                    # even head: V cols 0:128 -> O rows 0:64, sums 64:128
                    # odd  head: V cols 64:192 -> sums 0:64, O rows 64:128
                    voff = 64 * e
                    olo, ohi = (0, 64) if e == 0 else (64, 128)
                    slo, shi = (64, 128) if e == 0 else (0, 64)
                    ps = psAV.tile([128, 512], F32, tag="psAV")
                    for s in range(SC):
                        nc.tensor.matmul(
                            ps[:], VA[s][:, hp, voff:voff + 128],
                            ex[e][s][:, rr],
                            start=(s == 0), stop=(s == SC - 1))
                    nc.vector.tensor_copy(OT[hp][olo:ohi, rr],
                                          ps[olo:ohi, :])
                    nc.vector.tensor_copy(sums_t[slo:shi, rc, hp, :],
                                          ps[slo:shi, :])

        # ============ head-streamed pipeline ============
        pending_ex = {}
        for hp in range(NP):
            q_proj(hp)
            k_proj(hp)
            pending_ex[hp] = attn_qk(hp)
            if hp == 0:
                v_proj(0)
            if hp == 2:
                v_proj(1)
            if hp >= 1:
                attn_av(hp - 1, pending_ex.pop(hp - 1))
        attn_av(NP - 1, pending_ex.pop(NP - 1))

        p_psAV.__exit__(None, None, None)
        p_psQK.__exit__(None, None, None)
        p_psAB.__exit__(None, None, None)
        p_exp.__exit__(None, None, None)

        # ============ batched softmax normalization ============
        p_rcpal = tc.tile_pool(name="rcpal", bufs=1)
        rcpalp = p_rcpal.__enter__()
        rcpal_t = rcpalp.tile([128, 2, NP, 512], F16, tag="rcpal")
        for rc in range(2):
            # in-place reciprocal over all 8 head pairs of this rc half
            _act_recip(nc, sums_t[:, rc, :, :], sums_t[:, rc, :, :])
            for hp in range(NP):
                # swap halves so each head's recip aligns with its O rows
                nc.gpsimd.dma_start(rcpal_t[0:64, rc, hp, :],
                                    sums_t[64:128, rc, hp, :])
                nc.gpsimd.dma_start(rcpal_t[64:128, rc, hp, :],
                                    sums_t[0:64, rc, hp, :])
            rr = slice(rc * 512, rc * 512 + 512)
            for hp in range(NP):
                nc.vector.tensor_mul(OT[hp][:, rr], OT[hp][:, rr],
                                     rcpal_t[:, rc, hp, :])

        # ================= output projection =================
        p_psD = tc.tile_pool(name="psD", bufs=5, space="PSUM")
        psD = p_psD.__enter__()
        p_y = tc.tile_pool(name="y", bufs=4)
        yp = p_y.__enter__()
        for rp in range(KC):
            for n in range(2):
                ps = psD.tile([128, 512], F32, tag="psD")
                for k in range(KC):
                    nc.tensor.matmul(
                        ps[:], OT[k][:, rp * 128:(rp + 1) * 128],
                        wp_t[k][:, n * 512:(n + 1) * 512],
                        start=(k == 0), stop=(k == KC - 1))
                yt = yp.tile([128, 512], F32, tag="y")
                nc.vector.tensor_add(yt[:], ps[:], bp_t[:, n * 512:(n + 1) * 512])
                eng = nc.sync if (rp * 2 + n) % 2 == 0 else nc.gpsimd
                eng.dma_start(
                    y[rp * 128:(rp + 1) * 128, n * 512:(n + 1) * 512], yt[:])
        p_y.__exit__(None, None, None)
        p_psD.__exit__(None, None, None)
        p_rcpal.__exit__(None, None, None)
        p_ld.__exit__(None, None, None)
        p_sums.__exit__(None, None, None)
        p_wp.__exit__(None, None, None)
        p_ot.__exit__(None, None, None)
        p_qt.__exit__(None, None, None)
        p_kv.__exit__(None, None, None)
        p_const.__exit__(None, None, None)

    nc.compile()
    return nc


def _get_nc():
    if "nc" not in _CACHE:
        _CACHE["nc"] = _build()
    return _CACHE["nc"]


def kernel(x, context, context_mask, Wq, bq, Wkv, bkv, Wp, bp):
    global last_results
    x = np.asarray(x, dtype=np.float32)
    context = np.asarray(context, dtype=np.float32)
    context_mask = np.asarray(context_mask)
    Wq = np.asarray(Wq, dtype=np.float32)
    bq = np.asarray(bq, dtype=np.float32)
    Wkv = np.asarray(Wkv, dtype=np.float32)
    bkv = np.asarray(bkv, dtype=np.float32)
    Wp = np.asarray(Wp, dtype=np.float32)
    bp = np.asarray(bp, dtype=np.float32)

    sc = 1.0 / np.sqrt(HD)
    # kv reshape in the reference is [S, 2, H, Hd]: k cols = Wkv[:, :D]
    wq_h = np.ascontiguousarray((Wq * sc).astype(np.float16))
    bq_h = np.ascontiguousarray((bq * sc).reshape(KC, 128).T)
    wk_h = np.ascontiguousarray(Wkv[:, :D].astype(np.float16))
    bk_h = np.ascontiguousarray(bkv[:D].reshape(KC, 128).T)
    wv_h = np.ascontiguousarray((Wkv[:, D:] * 2.0**-10).astype(np.float16))
    bv = bkv[D:]
    wp_h = np.ascontiguousarray(Wp.astype(np.float16))
    bp_eff = bp + bv @ Wp          # softmax rows sum to 1
    bp_r = np.ascontiguousarray(
        np.broadcast_to(bp_eff.astype(np.float32), (128, D)))
    ones_h = np.full((128, 512), 2.0**-10, dtype=np.float16)

    in_maps = []
    for c in range(NCORE):
        b = c // 2
        r0 = (c % 2) * R
        in_maps.append({
            "xT": np.ascontiguousarray(x[b, r0:r0 + R, :].T.astype(np.float16)),
            "ctxT": np.ascontiguousarray(context[b].T.astype(np.float16)),
            "maskb": np.ascontiguousarray(
                np.where(context_mask[b], 0.0, NEG).astype(np.float32)
                .reshape(SC, 128).T),
            "wq": wq_h, "bq": bq_h,
            "wk": wk_h, "bk": bk_h,
            "wv": wv_h,
            "wp": wp_h, "bp_r": bp_r, "ones": ones_h,
        })

    nc = _get_nc()
    res = run_bass_kernel_spmd(nc, in_maps, list(range(NCORE)),
                               trace=bool(os.environ.get("BASS_TRACE")))
    last_results = res

    out = np.empty((B, T, D), dtype=np.float32)
    for c in range(NCORE):
        b = c // 2
        r0 = (c % 2) * R
        out[b, r0:r0 + R, :] = res.results[c]["y"]
    return out


# revision 9
# speedup vs baseline: 1.0275x; 1.0275x over previous
"""Cross-attention kernel for TRN2, 8-core SPMD.

Reference op (B=4, T=2048, S=512, D=1024, H=16, Hd=64):
    q = (x @ Wq + bq); k,v = context @ Wkv + bkv
    out = softmax(q k^T / sqrt(Hd) + mask) @ v @ Wp + bp

Sharding: pure data-parallel over (batch, T/2): core c owns batch c//2,
query rows (c%2)*1024..+1024.  Each core recomputes K/V for its batch
(2x duplicated KV-proj work, zero collectives).  Weights replicated.

v2 schedule (per core, R=1024 query rows).  The ACT exp stream (64 x
[128,1024] Exp, ~1.1us each) is the pacing engine, so the kernel is
restructured as a per-head-pair software pipeline that starts it ASAP
and never blocks it:
  - head-streamed: q_proj(m) -> k_proj(m) -> QK(m) + exps, with
    av(m-1) and the two v_proj halves woven in, so the first Exp fires
    ~10us in (vs ~51us for phase-ordered) and ACT runs continuously.
  - ALL softmax reciprocals are deferred to one batched ACT phase after
    the last Exp (sums staged to SBUF in fp16): one Exp->Recip LUT
    switch total, instead of 12 table loads thrashing mid-stream (each
    1.3us, and the blockage used to idle the PE long enough for HAM to
    re-throttle it to 1.2GHz for 24us).
  - input DMAs spread over all three DGE queues (sync/SP, scalar/ACT,
    gpsimd) so q_proj's operands land by ~8us; y output DMAs alternate
    sync/gpsimd to halve the drain tail.

Device design otherwise as v1: all activations flow transposed
(feature on partitions) so contractions land on partitions with no
on-chip transposes (host pre-transposes x/context):
  - KT [D,S], QT [D,R] fp16; V_aug fp16 [S, 8 pairs x 192] =
    [V_even(64)|ones(64)|V_odd(64)], wv and ones pre-scaled 2^-10 so
    unnormalized attention outputs fit fp16.
  - scores^T [S,R] per head: K=64 fp16 matmuls, two heads in PE
    row-groups 0/64 (concurrent), one ACT Exp per [128,1024] tile with
    the context mask as per-partition bias; no max-subtraction
    (|scores| <= ~8; fp32 exp is safe).
  - AV: fp16 matmuls -> psum [128,512]; the ones columns emit the
    softmax denominators for free; O and sums staged to SBUF by DVE
    (sums in fp16), batched ACT Reciprocal in-place at the end, a
    cross-partition DMA aligns each head's recip with its O rows, one
    DVE multiply normalizes OT.
  - Y [R,D] = OT^T @ Wp + bp, evictions + DMAs interleaved, rc0 rows
    first (their normalize completes first).

Numerics: fp16 operands (N(0,1)-scaled data: fp16 beats bf16 ~8x in
precision at identical PE throughput); PSUM accumulation fp32.
Max-abs error vs fp32 reference ~1e-3 of max|out|.
"""
import os
import sys
import types

import numpy as np

import concourse.tile as tile
from concourse import bacc, mybir
from concourse.bass_utils import run_bass_kernel_spmd

F32 = mybir.dt.float32
F16 = mybir.dt.float16
AF = mybir.ActivationFunctionType

B, T, S, D = 4, 2048, 512, 1024
H, HD = 16, 64
NCORE = 8
R = B * T // NCORE          # 1024 query rows per core
KC = D // 128               # 8 contraction chunks
SC = S // 128               # 4 context chunks
NP = H // 2                 # 8 head pairs
NEG = -60.0                 # mask bias (exp(-60) ~ 0)

_CACHE = {}
last_results = None         # BassKernelResults of the most recent run


def _install_ntff_hook():
    """antenv.axon_hooks is absent in this image; recreate it from the
    boot helper so BASS_TRACE=1 profiling works. Best-effort."""
    try:
        import antenv.axon_hooks  # noqa: F401
        return
    except ImportError:
        pass
    try:
        from trn_agent_boot.trn_boot import _ntff_profile_via_ctypes
        hook = _ntff_profile_via_ctypes("/opt/axon/libaxon_pjrt.so")
        mod = types.ModuleType("antenv.axon_hooks")
        mod.get_axon_ntff_profile_hook = lambda: hook
        sys.modules["antenv.axon_hooks"] = mod
    except Exception:
        pass


_install_ntff_hook()


def _act_recip(nc, out_ap, in_ap):
    """Raw ACT Reciprocal (bass blocks the helper for accuracy reasons;
    measured ~1e-5 rel err here, plenty for softmax denominators)."""
    eng = nc.scalar
    return eng.add_instruction(
        mybir.InstActivation(
            name=nc.get_next_instruction_name(),
            func=AF.Reciprocal,
            ins=[eng.lower_ap(in_ap),
                 mybir.ImmediateValue(dtype=F32, value=0.0),
                 mybir.ImmediateValue(dtype=F32, value=1.0),
                 mybir.ImmediateValue(dtype=F32, value=0.0)],
            outs=[eng.lower_ap(out_ap)],
        ))


def _build():
    nc = bacc.Bacc("TRN2", target_bir_lowering=False, debug=False,
                   num_devices=NCORE)

    xT = nc.dram_tensor("xT", [D, R], F16, kind="ExternalInput").ap()
    ctxT = nc.dram_tensor("ctxT", [D, S], F16, kind="ExternalInput").ap()
    maskb = nc.dram_tensor("maskb", [128, SC], F32, kind="ExternalInput").ap()
    wq = nc.dram_tensor("wq", [D, D], F16, kind="ExternalInput").ap()
    bq = nc.dram_tensor("bq", [128, KC], F32, kind="ExternalInput").ap()
    wk = nc.dram_tensor("wk", [D, D], F16, kind="ExternalInput").ap()
    bk = nc.dram_tensor("bk", [128, KC], F32, kind="ExternalInput").ap()
    wv = nc.dram_tensor("wv", [D, D], F16, kind="ExternalInput").ap()
    wp = nc.dram_tensor("wp", [D, D], F16, kind="ExternalInput").ap()
    bp_r = nc.dram_tensor("bp_r", [128, D], F32, kind="ExternalInput").ap()
    ones = nc.dram_tensor("ones", [128, 512], F16, kind="ExternalInput").ap()
    y = nc.dram_tensor("y", [R, D], F32, kind="ExternalOutput").ap()

    with tile.TileContext(nc) as tc:
        # Pool stack bottom -> top.  exp/psum pools sit on top so they
        # can close after av(7), making room for rcpal / psD / y.
        p_const = tc.tile_pool(name="const", bufs=1)
        p_kv = tc.tile_pool(name="kv", bufs=1)
        p_qt = tc.tile_pool(name="qt", bufs=1)
        p_ot = tc.tile_pool(name="ot", bufs=1)
        p_wp = tc.tile_pool(name="wpp", bufs=1)
        p_sums = tc.tile_pool(name="sums", bufs=1)
        p_ld = tc.tile_pool(name="ld", bufs=1)
        p_exp = tc.tile_pool(name="exp", bufs=18)
        p_psAB = tc.tile_pool(name="psAB", bufs=2, space="PSUM")
        p_psQK = tc.tile_pool(name="psQK", bufs=2, space="PSUM")
        p_psAV = tc.tile_pool(name="psAV", bufs=2, space="PSUM")
        constp = p_const.__enter__()
        kvp = p_kv.__enter__()
        qtp = p_qt.__enter__()
        otp = p_ot.__enter__()
        wpp = p_wp.__enter__()
        sumsp = p_sums.__enter__()
        ldp = p_ld.__enter__()
        expp = p_exp.__enter__()
        psAB = p_psAB.__enter__()
        psQK = p_psQK.__enter__()
        psAV = p_psAV.__enter__()

        # ---- PE warm-up on a memset tile: covers the initial DMA
        # window and gets HAM to K=8/8 before real work ----
        warm_sb = constp.tile([128, 512], F16, tag="warm_sb")
        nc.vector.memset(warm_sb[:], 0.0)
        warm_ps = psAB.tile([128, 512], F32, tag="psAB")
        for w in range(20):
            nc.tensor.matmul(warm_ps[:], warm_sb[:, 0:128], warm_sb[:],
                             start=True, stop=True, skip_group_check=True)

        # ---- input DMAs, spread over the 3 DGE queues; ordered so
        # q_proj(0)/k_proj(0) operands land first ----
        xT_t = [ldp.tile([128, R], F16, tag=f"xT{k}", name=f"xTs{k}")
                for k in range(KC)]
        wq_t = [ldp.tile([128, D], F16, tag=f"wq{k}", name=f"wqs{k}")
                for k in range(KC)]
        wk_t = [ldp.tile([128, D], F16, tag=f"wk{k}", name=f"wk{k}")
                for k in range(KC)]
        wv_t = [ldp.tile([128, D], F16, tag=f"wv{k}", name=f"wv{k}")
                for k in range(KC)]
        ctx_t = [ldp.tile([128, S], F16, tag=f"ctx{k}", name=f"ctx{k}")
                 for k in range(KC)]
        mb_t = constp.tile([128, SC], F32, tag="mb")
        bq_t = constp.tile([128, KC], F32, tag="bq")
        bk_t = constp.tile([128, KC], F32, tag="bk")
        bp_t = constp.tile([128, D], F32, tag="bp")
        wp_t = [wpp.tile([128, D], F16, tag=f"wp{k}", name=f"wps{k}")
                for k in range(KC)]

        # ---- persistent attention operands (fp16) ----
        KT = [kvp.tile([128, S], F16, tag=f"KT{m}", name=f"KT{m}")
              for m in range(KC)]
        # V_aug: [128, pair, 192] = [V_even | ones(64) | V_odd]
        VA = [kvp.tile([128, NP, 192], F16, tag=f"VA{s}", name=f"VA{s}")
              for s in range(SC)]

        # Input DMAs: each DGE queue sustains ~150GB/s, so order chunks
        # by compute deadline: (wk,ctx) for k_proj first, then (wq,xT)
        # for q_proj, then wv / wp / bp.
        # sync/SP queue: wk[0:4], wq, ones->VA, wp
        for k in range(4):
            nc.sync.dma_start(wk_t[k][:], wk[k * 128:(k + 1) * 128, :])
        for k in range(KC):
            nc.sync.dma_start(wq_t[k][:], wq[k * 128:(k + 1) * 128, :])
        for s in range(SC):
            nc.sync.dma_start(VA[s][:, :, 64:128],
                              ones[:].rearrange("p (h c) -> p h c", c=64))
        for k in range(KC):
            nc.sync.dma_start(wp_t[k][:], wp[k * 128:(k + 1) * 128, :])
        # scalar/ACT queue: biases+mask (tiny), ctx; free from ~8us on
        # so the Exp stream is never queued behind a DMA
        nc.scalar.dma_start(bq_t[:], bq[:])
        nc.scalar.dma_start(bk_t[:], bk[:])
        nc.scalar.dma_start(mb_t[:], maskb[:])
        for k in range(KC):
            nc.scalar.dma_start(ctx_t[k][:], ctxT[k * 128:(k + 1) * 128, :])
        # gpsimd queue: wk[4:8], xT, wv, bp
        for k in range(4, KC):
            nc.gpsimd.dma_start(wk_t[k][:], wk[k * 128:(k + 1) * 128, :])
        for k in range(KC):
            nc.gpsimd.dma_start(xT_t[k][:], xT[k * 128:(k + 1) * 128, :])
        for k in range(KC):
            nc.gpsimd.dma_start(wv_t[k][:], wv[k * 128:(k + 1) * 128, :])
        nc.gpsimd.dma_start(bp_t[:], bp_r[:])
        QT = [qtp.tile([128, R], F16, tag=f"QT{m}", name=f"QT{m}")
              for m in range(KC)]
        OT = [otp.tile([128, R], F16, tag=f"OT{m}", name=f"OT{m}")
              for m in range(KC)]
        # sums / rcp staging: [128, rc, hp, 512] fp16
        sums_t = sumsp.tile([128, 2, NP, 512], F16, tag="sums")

        def q_proj(m):
            for rc in range(2):
                ps = psAB.tile([128, 512], F32, tag="psAB")
                for k in range(KC):
                    nc.tensor.matmul(
                        ps[:], wq_t[k][:, m * 128:(m + 1) * 128],
                        xT_t[k][:, rc * 512:(rc + 1) * 512],
                        start=(k == 0), stop=(k == KC - 1))
                nc.vector.tensor_scalar_add(
                    QT[m][:, rc * 512:(rc + 1) * 512], ps[:], bq_t[:, m:m + 1])

        def k_proj(m):
            ps = psAB.tile([128, S], F32, tag="psAB")
            for k in range(KC):
                nc.tensor.matmul(ps[:], wk_t[k][:, m * 128:(m + 1) * 128],
                                 ctx_t[k][:],
                                 start=(k == 0), stop=(k == KC - 1))
            nc.vector.tensor_scalar_add(KT[m][:], ps[:], bk_t[:, m:m + 1])

        def v_proj(n):
            for s in range(SC):
                ps = psAB.tile([128, 512], F32, tag="psAB")
                for k in range(KC):
                    nc.tensor.matmul(ps[:], ctx_t[k][:, s * 128:(s + 1) * 128],
                                     wv_t[k][:, n * 512:(n + 1) * 512],
                                     start=(k == 0), stop=(k == KC - 1))
                # scatter 8 heads (4 pairs) into V_aug blocks
                src = ps[:].rearrange("p (h c) -> p h c", c=64)
                nc.vector.tensor_copy(VA[s][:, 4 * n:4 * n + 4, 0:64],
                                      src[:, 0::2, :])
                nc.vector.tensor_copy(VA[s][:, 4 * n:4 * n + 4, 128:192],
                                      src[:, 1::2, :])

        def attn_qk(hp):
            ex = [[expp.tile([128, R], F16, tag="exp", name=f"ex{hp}_{e}_{s}")
                   for s in range(SC)] for e in range(2)]
            for s in range(SC):
                # interleave the two head row-groups so the PE runs the
                # K=64 matmuls concurrently in row-group tiles
                pss = [psQK.tile([128, R], F32, tag="psQK",
                                 name=f"psqk{hp}_{s}_{e}") for e in range(2)]
                for rc in range(2):
                    for e in range(2):
                        lo, hi = 64 * e, 64 * e + 64
                        nc.tensor.matmul(
                            pss[e][:, rc * 512:(rc + 1) * 512],
                            KT[hp][lo:hi, s * 128:(s + 1) * 128],
                            QT[hp][lo:hi, rc * 512:(rc + 1) * 512],
                            start=True, stop=True)
                for e in range(2):
                    nc.scalar.activation(ex[e][s][:], pss[e][:],
                                         AF.Exp, bias=mb_t[:, s:s + 1])
            return ex

        def attn_av(hp, ex):
            for rc in range(2):
                rr = slice(rc * 512, rc * 512 + 512)
                for e in range(2):
                    # even head: V cols 0:128 -> O rows 0:64, sums 64:128
                    # odd  head: V cols 64:192 -> sums 0:64, O rows 64:128
                    voff = 64 * e
                    olo, ohi = (0, 64) if e == 0 else (64, 128)
                    slo, shi = (64, 128) if e == 0 else (0, 64)
                    ps = psAV.tile([128, 512], F32, tag="psAV")
                    for s in range(SC):
                        nc.tensor.matmul(
                            ps[:], VA[s][:, hp, voff:voff + 128],
                            ex[e][s][:, rr],
                            start=(s == 0), stop=(s == SC - 1))
                    nc.vector.tensor_copy(OT[hp][olo:ohi, rr],
                                          ps[olo:ohi, :])
                    nc.vector.tensor_copy(sums_t[slo:shi, rc, hp, :],
                                          ps[slo:shi, :])

        # ============ head-streamed pipeline ============
        # K-proj leads (its inputs land first at ~150GB/s/queue); q/qk
        # blocks start once xT/wq arrive; v_proj and av(hp-1) fill the
        # PE while the ACT Exp stream paces the attention inner loop.
        pending_ex = {}
        for m in range(7):
            k_proj(m)
        for hp in range(NP):
            q_proj(hp)
            if hp == 1:
                k_proj(7)
            if hp == 2:
                v_proj(0)
            if hp == 4:
                v_proj(1)
            pending_ex[hp] = attn_qk(hp)
            if hp >= 2:
                attn_av(hp - 2, pending_ex.pop(hp - 2))
        for hp in range(NP - 2, NP):
            attn_av(hp, pending_ex.pop(hp))

        p_psAV.__exit__(None, None, None)
        p_psQK.__exit__(None, None, None)
        p_psAB.__exit__(None, None, None)
        p_exp.__exit__(None, None, None)

        # ============ batched softmax normalization ============
        p_rcpal = tc.tile_pool(name="rcpal", bufs=1)
        rcpalp = p_rcpal.__enter__()
        rcpal_t = rcpalp.tile([128, 2, NP, 512], F16, tag="rcpal")
        for rc in range(2):
            # in-place reciprocal over all 8 head pairs of this rc half
            _act_recip(nc, sums_t[:, rc, :, :], sums_t[:, rc, :, :])
            # swap halves (one batched DMA per half per queue) so each
            # head's recip aligns with its O rows
            eng = nc.gpsimd if rc == 0 else nc.sync
            eng.dma_start(rcpal_t[0:64, rc, :, :], sums_t[64:128, rc, :, :])
            eng.dma_start(rcpal_t[64:128, rc, :, :], sums_t[0:64, rc, :, :])
            rr = slice(rc * 512, rc * 512 + 512)
            for hp in range(NP):
                nc.vector.tensor_mul(OT[hp][:, rr], OT[hp][:, rr],
                                     rcpal_t[:, rc, hp, :])

        # ================= output projection =================
        p_psD = tc.tile_pool(name="psD", bufs=5, space="PSUM")
        psD = p_psD.__enter__()
        p_y = tc.tile_pool(name="y", bufs=4)
        yp = p_y.__enter__()
        for rp in range(KC):
            for n in range(2):
                ps = psD.tile([128, 512], F32, tag="psD")
                for k in range(KC):
                    nc.tensor.matmul(
                        ps[:], OT[k][:, rp * 128:(rp + 1) * 128],
                        wp_t[k][:, n * 512:(n + 1) * 512],
                        start=(k == 0), stop=(k == KC - 1))
                yt = yp.tile([128, 512], F32, tag="y")
                nc.vector.tensor_add(yt[:], ps[:], bp_t[:, n * 512:(n + 1) * 512])
                eng = nc.sync if (rp * 2 + n) % 2 == 0 else nc.gpsimd
                eng.dma_start(
                    y[rp * 128:(rp + 1) * 128, n * 512:(n + 1) * 512], yt[:])
        p_y.__exit__(None, None, None)
        p_psD.__exit__(None, None, None)
        p_rcpal.__exit__(None, None, None)
        p_ld.__exit__(None, None, None)
        p_sums.__exit__(None, None, None)
        p_wp.__exit__(None, None, None)
        p_ot.__exit__(None, None, None)
        p_qt.__exit__(None, None, None)
        p_kv.__exit__(None, None, None)
        p_const.__exit__(None, None, None)

    nc.compile()
    return nc


def _get_nc():
    if "nc" not in _CACHE:
        _CACHE["nc"] = _build()
    return _CACHE["nc"]


def kernel(x, context, context_mask, Wq, bq, Wkv, bkv, Wp, bp):
    global last_results
    x = np.asarray(x, dtype=np.float32)
    context = np.asarray(context, dtype=np.float32)
    context_mask = np.asarray(context_mask)
    Wq = np.asarray(Wq, dtype=np.float32)
    bq = np.asarray(bq, dtype=np.float32)
    Wkv = np.asarray(Wkv, dtype=np.float32)
    bkv = np.asarray(bkv, dtype=np.float32)
    Wp = np.asarray(Wp, dtype=np.float32)
    bp = np.asarray(bp, dtype=np.float32)

    sc = 1.0 / np.sqrt(HD)
    # kv reshape in the reference is [S, 2, H, Hd]: k cols = Wkv[:, :D]
    wq_h = np.ascontiguousarray((Wq * sc).astype(np.float16))
    bq_h = np.ascontiguousarray((bq * sc).reshape(KC, 128).T)
    wk_h = np.ascontiguousarray(Wkv[:, :D].astype(np.float16))
    bk_h = np.ascontiguousarray(bkv[:D].reshape(KC, 128).T)
    wv_h = np.ascontiguousarray((Wkv[:, D:] * 2.0**-10).astype(np.float16))
    bv = bkv[D:]
    wp_h = np.ascontiguousarray(Wp.astype(np.float16))
    bp_eff = bp + bv @ Wp          # softmax rows sum to 1
    bp_r = np.ascontiguousarray(
        np.broadcast_to(bp_eff.astype(np.float32), (128, D)))
    ones_h = np.full((128, 512), 2.0**-10, dtype=np.float16)

    in_maps = []
    for c in range(NCORE):
        b = c // 2
        r0 = (c % 2) * R
        in_maps.append({
            "xT": np.ascontiguousarray(x[b, r0:r0 + R, :].T.astype(np.float16)),
            "ctxT": np.ascontiguousarray(context[b].T.astype(np.float16)),
            "maskb": np.ascontiguousarray(
                np.where(context_mask[b], 0.0, NEG).astype(np.float32)
                .reshape(SC, 128).T),
            "wq": wq_h, "bq": bq_h,
            "wk": wk_h, "bk": bk_h,
            "wv": wv_h,
            "wp": wp_h, "bp_r": bp_r, "ones": ones_h,
        })

    nc = _get_nc()
    res = run_bass_kernel_spmd(nc, in_maps, list(range(NCORE)),
                               trace=bool(os.environ.get("BASS_TRACE")))
    last_results = res

    out = np.empty((B, T, D), dtype=np.float32)
    for c in range(NCORE):
        b = c // 2
        r0 = (c % 2) * R
        out[b, r0:r0 + R, :] = res.results[c]["y"]
    return out
